# revision 1
# baseline (speedup 1.0000x reference)
"""Trainium2 Bass kernel for nn_CortexReasoner (masked-update attention with
Iron RoPE + relative Fourier bias).

Sharding: one attention head per NeuronCore (n_head == n_cores == 8), both
batches on every core; the output projection is redistributed with an
All-to-All so each core finalizes a disjoint 512-row slice of the output.

Host-side work is limited to layout prep and tiny trig tables (O(B*T*M)):
all matmuls over x / attention / projections run on device in fp32r.
"""

import math
import os
import sys

import numpy as np

for _p in ("/opt/trn_rl_repo",):
    if _p not in sys.path and os.path.isdir(_p):
        sys.path.append(_p)

import concourse.bass as bass
import concourse.mybir as mybir
import concourse.tile as tile
from concourse.bass_utils import run_bass_kernel_spmd
F32 = mybir.dt.float32
F32R = mybir.dt.float32r
AF = mybir.ActivationFunctionType

B, T, D = 2, 2048, 1024
H = 8
HD = 128          # head dim
N_CORES = 8
BT = B * T        # 4096
ROWS = BT // N_CORES   # 512 output rows per core
NCH = 8           # t-chunks of 512 across B*T
CT = D // 128     # 8 contraction tiles for the projections
KT = T // 128     # 16 key tiles per batch
QC = T // 512     # 4 query chunks per batch


def _build_nc():
    nc = bass.Bass()

    xT = nc.dram_tensor("xT", [D, BT], F32, kind="ExternalInput")
    wq = nc.dram_tensor("wq", [D, HD], F32, kind="ExternalInput")
    wk = nc.dram_tensor("wk", [D, HD], F32, kind="ExternalInput")
    wv = nc.dram_tensor("wv", [D, HD], F32, kind="ExternalInput")
    bq = nc.dram_tensor("bq", [HD, 1], F32, kind="ExternalInput")
    bk = nc.dram_tensor("bk", [HD, 1], F32, kind="ExternalInput")
    bv = nc.dram_tensor("bv", [HD, 1], F32, kind="ExternalInput")
    At = nc.dram_tensor("At", [B, 128, T], F32, kind="ExternalInput")    # [cos;cos]
    Bt = nc.dram_tensor("Bt", [B, 128, T], F32, kind="ExternalInput")    # [-sin;sin]
    fk = nc.dram_tensor("fk", [B, 64, T], F32, kind="ExternalInput")
    fq = nc.dram_tensor("fq", [B, 64, T], F32, kind="ExternalInput")
    pswp = nc.dram_tensor("pswp", [128, 128], F32, kind="ExternalInput")
    identity = nc.dram_tensor("identity", [128, 128], F32, kind="ExternalInput")
    ones128 = nc.dram_tensor("ones128", [128, 1], F32, kind="ExternalInput")
    ones1 = nc.dram_tensor("ones1", [1, 128], F32, kind="ExternalInput")
    wo = nc.dram_tensor("wo", [D, D], F32, kind="ExternalInput")
    maskc = nc.dram_tensor("maskc", [ROWS, 1], F32, kind="ExternalInput")
    in1m = nc.dram_tensor("in1m", [ROWS, D], F32, kind="ExternalInput")

    out = nc.dram_tensor("out", [ROWS, D], F32, kind="ExternalOutput")

    with tile.TileContext(nc) as tc, \
         nc.allow_low_precision(reason="fp32r matmul pipeline"):
        with tc.tile_pool(name="persist", bufs=1) as pp, \
             tc.tile_pool(name="consts", bufs=1) as cp, \
             tc.tile_pool(name="dram", bufs=1, space="DRAM") as dp:

            qrot = [pp.tile([128, T], F32R, tag=f"qrot{b}", name=f"qrot{b}") for b in range(B)]
            krot = [pp.tile([128, T], F32R, tag=f"krot{b}", name=f"krot{b}") for b in range(B)]
            vnat = [pp.tile([128, 16 * 128], F32R, tag=f"vnat{b}", name=f"vnat{b}") for b in range(B)]

            tP = cp.tile([128, 128], F32R)
            t1s = cp.tile([128, 1], F32R)
            t1r = cp.tile([1, 128], F32R)
            ident = cp.tile([128, 128], F32R)
            nc.sync.dma_start(out=tP[:], in_=pswp.bitcast(F32R)[:])
            nc.sync.dma_start(out=t1s[:], in_=ones128.bitcast(F32R)[:])
            nc.sync.dma_start(out=t1r[:], in_=ones1.bitcast(F32R)[:])
            nc.sync.dma_start(out=ident[:], in_=identity.bitcast(F32R)[:])

            # ---------------- Phase 1: QKV projection + RoPE + V transpose
            with tc.tile_pool(name="ph1", bufs=1) as p1, \
                 tc.tile_pool(name="ph1x", bufs=16) as p1x, \
                 tc.tile_pool(name="ph1s", bufs=2) as p1s, \
                 tc.tile_pool(name="ph1t", bufs=4) as p1t, \
                 tc.tile_pool(name="ps1", bufs=4, space="PSUM") as ps1, \
                 tc.tile_pool(name="ps1b", bufs=2, space="PSUM") as ps1b, \
                 tc.tile_pool(name="ps1c", bufs=1, space="PSUM") as ps1c:

                wqt = p1.tile([128, CT * HD], F32R)
                wkt = p1.tile([128, CT * HD], F32R)
                wvt = p1.tile([128, CT * HD], F32R)
                for ct in range(CT):
                    s = slice(ct * HD, (ct + 1) * HD)
                    nc.sync.dma_start(out=wqt[:, s], in_=wq.bitcast(F32R)[ct * 128:(ct + 1) * 128, :])
                    nc.sync.dma_start(out=wkt[:, s], in_=wk.bitcast(F32R)[ct * 128:(ct + 1) * 128, :])
                    nc.sync.dma_start(out=wvt[:, s], in_=wv.bitcast(F32R)[ct * 128:(ct + 1) * 128, :])
                tbq = p1.tile([128, 1], F32)
                tbk = p1.tile([128, 1], F32)
                tbv = p1.tile([128, 1], F32)
                nc.sync.dma_start(out=tbq[:], in_=bq[:])
                nc.sync.dma_start(out=tbk[:], in_=bk[:])
                nc.sync.dma_start(out=tbv[:], in_=bv[:])
                tAt = []
                tBt = []
                for b in range(B):
                    a_b = p1.tile([128, T], F32, tag=f"At{b}")
                    b_b = p1.tile([128, T], F32, tag=f"Bt{b}")
                    nc.sync.dma_start(out=a_b[:], in_=At[b])
                    nc.sync.dma_start(out=b_b[:], in_=Bt[b])
                    tAt.append(a_b)
                    tBt.append(b_b)

                for ch in range(NCH):
                    b = ch // (NCH // B)
                    tch = slice(ch * 512, (ch + 1) * 512)
                    tch_b = slice((ch % 4) * 512, (ch % 4 + 1) * 512)
                    xts = []
                    for ct in range(CT):
                        xt = p1x.tile([128, 512], F32R, tag="xt")
                        nc.sync.dma_start(out=xt[:], in_=xT.bitcast(F32R)[ct * 128:(ct + 1) * 128, tch])
                        xts.append(xt)
                    pq = ps1.tile([128, 512], F32, tag="pqkv")
                    pk = ps1.tile([128, 512], F32, tag="pqkv")
                    pv = ps1.tile([128, 512], F32, tag="pqkv")
                    for ct in range(CT):
                        st, sp = (ct == 0), (ct == CT - 1)
                        s = slice(ct * HD, (ct + 1) * HD)
                        nc.tensor.matmul(pq[:], wqt[:, s], xts[ct][:], start=st, stop=sp)
                        nc.tensor.matmul(pk[:], wkt[:, s], xts[ct][:], start=st, stop=sp)
                        nc.tensor.matmul(pv[:], wvt[:, s], xts[ct][:], start=st, stop=sp)

                    # q/k: add bias, rope-rotate into qrot/krot
                    for (psrc, tb, dstl) in ((pq, tbq, qrot), (pk, tbk, krot)):
                        dst = dstl[b]
                        sqk = p1s.tile([128, 512], F32R, tag="sqk")
                        nc.scalar.activation(sqk[:], psrc[:], AF.Identity, bias=tb[:])
                        psw = ps1b.tile([128, 512], F32, tag="psw")
                        nc.tensor.matmul(psw[:], tP[:], sqk[:], start=True, stop=True)
                        ta = p1t.tile([128, 512], F32, tag="ropeA")
                        tbm = p1t.tile([128, 512], F32, tag="ropeB")
                        nc.vector.tensor_mul(ta[:], sqk.bitcast(F32)[:], tAt[b][:, tch_b])
                        nc.vector.tensor_mul(tbm[:], psw[:], tBt[b][:, tch_b])
                        nc.vector.tensor_add(dst[:, tch_b], ta[:], tbm[:])

                    # v: bias then transpose 4x 128x128 into vnat
                    sv = p1s.tile([128, 512], F32R, tag="sv")
                    nc.scalar.activation(sv[:], pv[:], AF.Identity, bias=tbv[:])
                    for j in range(4):
                        ptr = ps1c.tile([128, 128], F32R, tag="ptr")
                        nc.tensor.transpose(ptr[:], sv[:, j * 128:(j + 1) * 128], ident[:])
                        g = (ch % 4) * 4 + j
                        nc.scalar.activation(vnat[b][:, g * 128:(g + 1) * 128], ptr.bitcast(F32)[:], AF.Copy)

            # rope tables come from DRAM directly in the muls above: pre-make
            # per-batch SBUF residency unnecessary (DVE reads DRAM? no) --
            # NOTE: DVE cannot read DRAM; see host-side: At/Bt are DMA'd below.

            # phase-3 prefetches: queue behind the phase-1 stream, drain during ph2
            two = cp.tile([128, CT * D], F32R)
            for dt in range(CT):
                nc.sync.dma_start(out=two[:, dt * D:(dt + 1) * D],
                                  in_=wo.bitcast(F32R)[dt * 128:(dt + 1) * 128, :])
            tmask = cp.tile([128, 4], F32)
            nc.sync.dma_start(out=tmask[:], in_=maskc.rearrange("(tt p) one -> p (tt one)", p=128))
            tin1 = cp.tile([128, 4 * D], F32)
            for tt in range(4):
                nc.sync.dma_start(out=tin1[:, tt * D:(tt + 1) * D],
                                  in_=in1m[tt * 128:(tt + 1) * 128, :])

            # ---------------- Phase 2: attention
            a2a_in = dp.tile([N_CORES, 128, 512], F32)
            a2a_out = dp.tile([N_CORES, 128, 512], F32)
            with tc.tile_pool(name="ph2c", bufs=1) as p2c, \
                 tc.tile_pool(name="ph2e", bufs=6) as p2e, \
                 tc.tile_pool(name="ph2r", bufs=2) as p2r, \
                 tc.tile_pool(name="ps2", bufs=4, space="PSUM") as ps2, \
                 tc.tile_pool(name="ps2y", bufs=2, space="PSUM") as ps2y, \
                 tc.tile_pool(name="ps2s", bufs=2, space="PSUM") as ps2s:

                tfk = []
                tfq = []
                for b in range(B):
                    fkb = p2c.tile([64, T], F32R)
                    fqb = p2c.tile([64, T], F32R)
                    nc.sync.dma_start(out=fkb[:], in_=fk.bitcast(F32R)[b])
                    nc.sync.dma_start(out=fqb[:], in_=fq.bitcast(F32R)[b])
                    tfk.append(fkb)
                    tfq.append(fqb)

                for u in range(B):
                    toff = u * T
                    for qc in range(QC):
                        qs = slice(qc * 512, (qc + 1) * 512)
                        py = ps2y.tile([128, 512], F32, tag="py")
                        psm = ps2s.tile([1, 512], F32, tag="psm")
                        pscs = {}
                        ses = {}
                        for kt in range(KT + 1):
                            if kt < KT:
                                psc = ps2.tile([128, 512], F32, tag="psc")
                                se = p2e.tile([128, 512], F32R, tag="exp")
                                pscs[kt] = psc
                                ses[kt] = se
                                ks = slice(kt * 128, (kt + 1) * 128)
                                nc.tensor.matmul(psc[:], krot[u][:, ks], qrot[u][:, qs], start=True, stop=False)
                                nc.tensor.matmul(psc[:], tfk[u][:, ks], tfq[u][:, qs], start=False, stop=True)
                                nc.scalar.activation(se[:], psc[:], AF.Exp)
                            if kt >= 1:
                                ktp = kt - 1
                                se = ses.pop(ktp)
                                nc.tensor.matmul(psm[:], t1s[:], se[:], start=(ktp == 0), stop=(ktp == KT - 1))
                                nc.tensor.matmul(py[:], vnat[u][:, ktp * 128:(ktp + 1) * 128], se[:],
                                                 start=(ktp == 0), stop=(ktp == KT - 1))
                        rr = p2r.tile([1, 512], F32, tag="rr")
                        nc.vector.reciprocal(rr[:], psm[:])
                        rd = dp.tile([1, 512], F32, tag="rdram", name="rdram", bufs=2)
                        nc.sync.dma_start(out=rd[:], in_=rr[:])
                        rb = p2r.tile([128, 512], F32, tag="rb")
                        rd_b = bass.AP(tensor=rd[:].tensor, offset=rd[:].offset,
                                       ap=[[0, 128], [1, 512]])
                        nc.sync.dma_start(out=rb[:], in_=rd_b)
                        ynrm = p2r.tile([128, 512], F32, tag="ynrm")
                        nc.vector.tensor_mul(ynrm[:], py[:], rb[:])
                        nc.sync.dma_start(out=a2a_in[u * QC + qc], in_=ynrm[:])

            # ---------------- Phase 3: A2A redistribute + output projection
            with tc.tile_pool(name="ph3", bufs=1) as p3, \
                 tc.tile_pool(name="ph3s", bufs=4) as p3s, \
                 tc.tile_pool(name="ps3", bufs=3, space="PSUM") as ps3:

                scratch = p3.tile([128, 512], F32R, name="scratch")
                nc.sync.dma_start(out=scratch[:], in_=a2a_in.bitcast(F32R)[0])
                pwarm = ps3.tile([128, 512], F32, tag="pwarm", name="pwarm")
                for wj in range(160):
                    nc.tensor.matmul(pwarm[:], scratch[:, 0:128], scratch[:],
                                     start=(wj == 0), stop=(wj == 159))
                nc.gpsimd.collective_compute(
                    "AllToAll", mybir.AluOpType.bypass,
                    ins=[a2a_in.opt()], outs=[a2a_out.opt()],
                    replica_groups=[list(range(N_CORES))],
                )
                ya = []
                for dt in range(N_CORES):
                    t = p3.tile([128, 512], F32R, tag=f"ya{dt}")
                    nc.sync.dma_start(out=t[:], in_=a2a_out.bitcast(F32R)[dt])
                    ya.append(t)
                for tt in range(4):
                    for nch in range(2):
                        po = ps3.tile([128, 512], F32, tag="po")
                        for dt in range(CT):
                            nc.tensor.matmul(po[:], ya[dt][:, tt * 128:(tt + 1) * 128],
                                             two[:, dt * D + nch * 512: dt * D + (nch + 1) * 512],
                                             start=(dt == 0), stop=(dt == CT - 1))
                        so = p3s.tile([128, 512], F32, tag="so")
                        nc.vector.scalar_tensor_tensor(
                            out=so[:], in0=po[:], scalar=tmask[:, tt:tt + 1],
                            in1=tin1[:, tt * D + nch * 512: tt * D + (nch + 1) * 512],
                            op0=mybir.AluOpType.mult, op1=mybir.AluOpType.add)
                        nc.sync.dma_start(out=out[tt * 128:(tt + 1) * 128, nch * 512:(nch + 1) * 512], in_=so[:])

    _split_multi_waits(nc)
    return nc


_ENGINES = None


def _split_multi_waits(nc):
    """This walrus build encodes at most one sync-wait per instruction; hoist
    extras onto preceding NoOps.  For the kernel-tail drain (many DMA-queue
    waits, followed by an all-engine barrier) spread the NoOps across all
    engines so the waits poll in parallel; elsewhere keep them on the same
    engine to preserve ordering semantics."""
    engs = [mybir.EngineType.SP, mybir.EngineType.Activation, mybir.EngineType.DVE,
            mybir.EngineType.PE, mybir.EngineType.Pool]
    for f in nc.m.functions:
        for bb in f.blocks:
            new_insts = []
            for inst in bb.instructions:
                si = inst.sync_info
                if si is not None and si.on_wait and len(si.on_wait) > 1:
                    waits = list(si.on_wait)
                    distribute = (type(inst).__name__ == "InstDrain"
                                  and len(waits) > 3)
                    for j, w in enumerate(waits[:-1]):
                        eng = engs[j % len(engs)] if distribute else inst.engine
                        new_insts.append(mybir.InstNoOp(
                            name=f"{inst.name}_wsplit{j}", ins=[], outs=[],
                            engine=eng,
                            sync_info=mybir.SyncInfo(on_wait=[w], on_update=[])))
                    si.on_wait = [waits[-1]]
                new_insts.append(inst)
            bb.instructions = new_insts


def _prep_inputs(x, coords, update_mask, Wqkv, bqkv, Wo, bo, W_rope, W_fb,
                 beta_cos, beta_sin):
    """Per-core input maps (host-side layout + tiny trig tables)."""
    f32 = np.float32
    x = np.asarray(x, f32)
    coords = np.asarray(coords, f32)
    update_mask = np.asarray(update_mask)
    Wqkv = np.asarray(Wqkv, f32)
    bqkv = np.asarray(bqkv, f32)
    Wo = np.ascontiguousarray(np.asarray(Wo, f32))
    bo = np.asarray(bo, f32)
    W_rope = np.asarray(W_rope, f32)
    W_fb = np.asarray(W_fb, f32)
    beta_cos = np.asarray(beta_cos, f32)
    beta_sin = np.asarray(beta_sin, f32)

    xf = x.reshape(BT, D)
    xT = np.ascontiguousarray(xf.T)

    # split-half channel order: evens then odds
    perm = np.concatenate([np.arange(0, HD, 2), np.arange(1, HD, 2)])
    inv_scale = f32(1.0 / math.sqrt(HD))

    # rope tables per batch: theta[m, t]; A=[cos;cos], B=[-sin;sin]
    At = np.empty((B, 128, T), f32)
    Bt = np.empty((B, 128, T), f32)
    fkT = np.empty((B, 64, T), f32)
    fqT = np.empty((B, 64, T), f32)
    for b in range(B):
        c1 = coords[b, :, 0].astype(np.float64)
        th = (W_rope[:, 0:1].astype(np.float64) * c1[None, :])
        cth = np.cos(th).astype(f32)
        sth = np.sin(th).astype(f32)
        At[b] = np.concatenate([cth, cth], axis=0)
        Bt[b] = np.concatenate([-sth, sth], axis=0)
        S = (W_fb[:, 0:1].astype(np.float64) * c1[None, :])
        cS = np.cos(S).astype(f32)
        sS = np.sin(S).astype(f32)
        fkT[b] = np.concatenate([cS, sS], axis=0)
        fqT[b] = np.concatenate([cS * beta_cos[:, None] + sS * beta_sin[:, None],
                                 sS * beta_cos[:, None] - cS * beta_sin[:, None]], axis=0)

    pswp = np.zeros((128, 128), f32)
    for i in range(128):
        pswp[(i + 64) % 128, i] = 1.0   # lhsT: (P^T x)[i] = x[(i+64)%128]

    ones128 = np.ones((128, 1), f32)
    ident_np = np.eye(128, dtype=f32)
    ones1 = np.ones((1, 128), f32)

    mask_f = update_mask.reshape(BT).astype(f32)

    in_maps = []
    for c in range(N_CORES):
        h = c
        wq_h = np.ascontiguousarray(Wqkv[:, h * HD:(h + 1) * HD][:, perm] * inv_scale)
        wk_h = np.ascontiguousarray(Wqkv[:, D + h * HD:D + (h + 1) * HD][:, perm])
        wv_h = np.ascontiguousarray(Wqkv[:, 2 * D + h * HD:2 * D + (h + 1) * HD])
        bq_h = (bqkv[h * HD:(h + 1) * HD][perm] * inv_scale).reshape(HD, 1)
        bk_h = bqkv[D + h * HD:D + (h + 1) * HD][perm].reshape(HD, 1)
        bv_h = bqkv[2 * D + h * HD:2 * D + (h + 1) * HD].reshape(HD, 1)
        rows = slice(c * ROWS, (c + 1) * ROWS)
        mrows = mask_f[rows].reshape(ROWS, 1)
        in1 = mrows * bo[None, :] + (1.0 - mrows) * xf[rows]
        in_maps.append(dict(
            xT=xT, wq=wq_h, wk=wk_h, wv=wv_h,
            bq=np.ascontiguousarray(bq_h), bk=np.ascontiguousarray(bk_h),
            bv=np.ascontiguousarray(bv_h),
            At=At, Bt=Bt, fk=fkT, fq=fqT, pswp=pswp,
            ones128=ones128, ones1=ones1, wo=Wo, identity=ident_np,
            maskc=np.ascontiguousarray(mrows),
            in1m=np.ascontiguousarray(in1),
        ))
    return in_maps


_NC_CACHE = None


def _get_nc():
    global _NC_CACHE
    if _NC_CACHE is None:
        _NC_CACHE = _build_nc()
    return _NC_CACHE


def run(trace=False, **inputs):
    nc = _get_nc()
    in_maps = _prep_inputs(**inputs)
    res = run_bass_kernel_spmd(nc, in_maps, core_ids=list(range(N_CORES)),
                               trace=trace)
    outs = [res.results[c]["out"] for c in range(N_CORES)]
    full = np.concatenate(outs, axis=0).reshape(B, T, D).astype(np.float32)
    return full, res


def kernel(**inputs) -> np.ndarray:
    full, _ = run(trace=False, **inputs)
    return full



# revision 3
# speedup vs baseline: 1.3136x; 1.3136x over previous
"""Trainium2 Bass kernel for nn_CortexReasoner (masked-update attention with
Iron RoPE + relative Fourier bias).

Sharding: one attention head per NeuronCore (n_head == n_cores == 8), both
batches on every core.  The output projection is redistributed with TWO
AllToAlls (one per batch) so the first one overlaps with the second batch's
attention compute; each core finalizes a 256-token slice of each batch.

All heavy operands travel as bf16 (PE runs bf16 matmuls at 1 cycle/row vs 2
for fp32r, and DMA bytes halve); accumulation stays fp32 in PSUM.
"""

import math
import os
import sys

import numpy as np
import ml_dtypes

for _p in ("/opt/trn_rl_repo",):
    if _p not in sys.path and os.path.isdir(_p):
        sys.path.append(_p)

import concourse.bass as bass
import concourse.mybir as mybir
import concourse.tile as tile
from concourse.bass_utils import run_bass_kernel_spmd

F32 = mybir.dt.float32
BF16 = mybir.dt.bfloat16
AF = mybir.ActivationFunctionType

B, T, D = 2, 2048, 1024
H = 8
HD = 128          # head dim
N_CORES = 8
BT = B * T        # 4096
ROWS = BT // N_CORES   # 512 output rows per core (256 from each batch)
NCH = 8           # t-chunks of 512 across B*T
CT = D // 128     # 8 contraction tiles for the projections
KT = T // 128     # 16 key tiles per batch
QC = T // 512     # 4 query chunks per batch
OWN = T // N_CORES     # 256 tokens owned per batch per core


def _build_nc():
    nc = bass.Bass()

    xT = nc.dram_tensor("xT", [D, BT], BF16, kind="ExternalInput")
    wq = nc.dram_tensor("wq", [128, CT * HD], BF16, kind="ExternalInput")
    wk = nc.dram_tensor("wk", [128, CT * HD], BF16, kind="ExternalInput")
    wv = nc.dram_tensor("wv", [128, CT * HD], BF16, kind="ExternalInput")
    bq = nc.dram_tensor("bq", [HD, 1], F32, kind="ExternalInput")
    bk = nc.dram_tensor("bk", [HD, 1], F32, kind="ExternalInput")
    bv = nc.dram_tensor("bv", [HD, 1], F32, kind="ExternalInput")
    At = nc.dram_tensor("At", [B, 128, T], BF16, kind="ExternalInput")    # [cos;cos]
    Bt = nc.dram_tensor("Bt", [B, 128, T], BF16, kind="ExternalInput")    # [-sin;sin]
    fk = nc.dram_tensor("fk", [B, 64, T], BF16, kind="ExternalInput")
    fq = nc.dram_tensor("fq", [B, 64, T], BF16, kind="ExternalInput")
    pswp = nc.dram_tensor("pswp", [128, 128], BF16, kind="ExternalInput")
    identity = nc.dram_tensor("identity", [128, 128], BF16, kind="ExternalInput")
    onesq = nc.dram_tensor("onesq", [128, 128], BF16, kind="ExternalInput")
    wo = nc.dram_tensor("wo", [128, CT * D], BF16, kind="ExternalInput")
    maskc = nc.dram_tensor("maskc", [ROWS, 1], F32, kind="ExternalInput")
    in1m = nc.dram_tensor("in1m", [ROWS, D], F32, kind="ExternalInput")

    out = nc.dram_tensor("out", [ROWS, D], F32, kind="ExternalOutput")

    with tile.TileContext(nc) as tc, \
         nc.allow_low_precision(reason="bf16 matmul pipeline"):
        with tc.tile_pool(name="persist", bufs=1) as pp, \
             tc.tile_pool(name="consts", bufs=1) as cp, \
             tc.tile_pool(name="dram", bufs=1, space="DRAM") as dp:

            qrot = [pp.tile([128, T], BF16, tag=f"qrot{b}", name=f"qrot{b}") for b in range(B)]
            krot = [pp.tile([128, T], BF16, tag=f"krot{b}", name=f"krot{b}") for b in range(B)]
            vnat = [pp.tile([128, KT * 128], BF16, tag=f"vnat{b}", name=f"vnat{b}") for b in range(B)]

            tP = cp.tile([128, 128], BF16)
            ident = cp.tile([128, 128], BF16)
            tones = cp.tile([128, 128], BF16)
            nc.sync.dma_start(out=tP[:], in_=pswp[:])
            nc.sync.dma_start(out=ident[:], in_=identity[:])
            nc.sync.dma_start(out=tones[:], in_=onesq[:])

            # collective staging (bf16 payloads), one pair per batch
            a2a_in = [dp.tile([N_CORES, 128, OWN], BF16, tag=f"a2ai{b}", name=f"a2ai{b}")
                      for b in range(B)]
            a2a_out = [dp.tile([N_CORES, 128, OWN], BF16, tag=f"a2ao{b}", name=f"a2ao{b}")
                       for b in range(B)]

            # deferred DMA issue schedule: chunk index -> list of thunks.
            # Critical-path-first: weights + chunk-0 x + batch-0 rope tables go
            # ahead of everything else; phase-2/3 operands stream in behind.
            tAt = [None] * B
            tBt = [None] * B
            tfk = [None] * B
            tfq = [None] * B
            two = cp.tile([128, CT * D], BF16)
            tmask = cp.tile([128, 4], F32)
            tin1 = cp.tile([128, 4 * D], F32)

            def load_rope(b):
                def _f():
                    a_b = cp.tile([128, T], BF16, tag=f"At{b}")
                    b_b = cp.tile([128, T], BF16, tag=f"Bt{b}")
                    nc.sync.dma_start(out=a_b[:], in_=At[b])
                    nc.sync.dma_start(out=b_b[:], in_=Bt[b])
                    tAt[b] = a_b
                    tBt[b] = b_b
                return _f

            def load_fourier(b):
                def _f():
                    fkb = cp.tile([64, T], BF16, tag=f"fk{b}")
                    fqb = cp.tile([64, T], BF16, tag=f"fq{b}")
                    nc.sync.dma_start(out=fkb[:], in_=fk[b])
                    nc.sync.dma_start(out=fqb[:], in_=fq[b])
                    tfk[b] = fkb
                    tfq[b] = fqb
                return _f

            def load_wo(half):
                def _f():
                    s = slice(half * CT * D // 2, (half + 1) * CT * D // 2)
                    nc.sync.dma_start(out=two[:, s], in_=wo[:, s])
                return _f

            def load_phase3_misc():
                nc.sync.dma_start(out=tmask[:], in_=maskc.rearrange("(tt p) one -> p (tt one)", p=128))
                for tt in range(4):
                    nc.sync.dma_start(out=tin1[:, tt * D:(tt + 1) * D],
                                      in_=in1m[tt * 128:(tt + 1) * 128, :])

            deferred = {
                0: [load_rope(0)],
                2: [load_rope(1)],
                4: [load_fourier(0)],
                5: [load_fourier(1)],
                6: [load_wo(0), load_wo(1)],
                7: [load_phase3_misc],
            }

            # ---------------- Phase 1: QKV projection + RoPE + V transpose
            with tc.tile_pool(name="ph1", bufs=1) as p1, \
                 tc.tile_pool(name="ph1x", bufs=16) as p1x, \
                 tc.tile_pool(name="ph1s", bufs=3) as p1s, \
                 tc.tile_pool(name="ph1t", bufs=4) as p1t, \
                 tc.tile_pool(name="ps1", bufs=4, space="PSUM") as ps1, \
                 tc.tile_pool(name="ps1b", bufs=2, space="PSUM") as ps1b, \
                 tc.tile_pool(name="ps1c", bufs=2, space="PSUM") as ps1c:

                wqt = p1.tile([128, CT * HD], BF16)
                wkt = p1.tile([128, CT * HD], BF16)
                wvt = p1.tile([128, CT * HD], BF16)
                nc.sync.dma_start(out=wqt[:], in_=wq[:])
                nc.sync.dma_start(out=wkt[:], in_=wk[:])
                nc.sync.dma_start(out=wvt[:], in_=wv[:])
                tbq = p1.tile([128, 1], F32)
                tbk = p1.tile([128, 1], F32)
                tbv = p1.tile([128, 1], F32)
                nc.sync.dma_start(out=tbq[:], in_=bq[:])
                nc.sync.dma_start(out=tbk[:], in_=bk[:])
                nc.sync.dma_start(out=tbv[:], in_=bv[:])

                for ch in range(NCH):
                    b = ch // (NCH // B)
                    tch = slice(ch * 512, (ch + 1) * 512)
                    tch_b = slice((ch % 4) * 512, (ch % 4 + 1) * 512)
                    xts = []
                    for ct in range(CT):
                        xt = p1x.tile([128, 512], BF16, tag="xt")
                        nc.sync.dma_start(out=xt[:], in_=xT[ct * 128:(ct + 1) * 128, tch])
                        xts.append(xt)
                    for fn in deferred.get(ch, []):
                        fn()
                    pq = ps1.tile([128, 512], F32, tag="pqkv")
                    pk = ps1.tile([128, 512], F32, tag="pqkv")
                    pv = ps1.tile([128, 512], F32, tag="pqkv")
                    for ct in range(CT):
                        st, sp = (ct == 0), (ct == CT - 1)
                        s = slice(ct * HD, (ct + 1) * HD)
                        nc.tensor.matmul(pq[:], wqt[:, s], xts[ct][:], start=st, stop=sp)
                        nc.tensor.matmul(pk[:], wkt[:, s], xts[ct][:], start=st, stop=sp)
                        nc.tensor.matmul(pv[:], wvt[:, s], xts[ct][:], start=st, stop=sp)

                    # q/k: add bias, rope-rotate into qrot/krot
                    for (psrc, tb, dstl) in ((pq, tbq, qrot), (pk, tbk, krot)):
                        dst = dstl[b]
                        sqk = p1s.tile([128, 512], BF16, tag="sqk")
                        nc.scalar.activation(sqk[:], psrc[:], AF.Identity, bias=tb[:])
                        psw = ps1b.tile([128, 512], F32, tag="psw")
                        nc.tensor.matmul(psw[:], tP[:], sqk[:], start=True, stop=True)
                        ta = p1t.tile([128, 512], BF16, tag="ropeA")
                        tbm = p1t.tile([128, 512], BF16, tag="ropeB")
                        nc.vector.tensor_mul(ta[:], sqk[:], tAt[b][:, tch_b])
                        nc.vector.tensor_mul(tbm[:], psw[:], tBt[b][:, tch_b])
                        nc.vector.tensor_add(dst[:, tch_b], ta[:], tbm[:])

                    # v: bias then transpose 4x 128x128 into vnat
                    sv = p1s.tile([128, 512], BF16, tag="sv")
                    nc.scalar.activation(sv[:], pv[:], AF.Identity, bias=tbv[:])
                    for j in range(4):
                        ptr = ps1c.tile([128, 128], BF16, tag="ptr")
                        nc.tensor.transpose(ptr[:], sv[:, j * 128:(j + 1) * 128], ident[:])
                        g = (ch % 4) * 4 + j
                        nc.scalar.activation(vnat[b][:, g * 128:(g + 1) * 128], ptr[:], AF.Copy)

            # ---------------- Phase 2: attention, one A2A per batch
            with tc.tile_pool(name="ph2e", bufs=6) as p2e, \
                 tc.tile_pool(name="ph2r", bufs=2) as p2r, \
                 tc.tile_pool(name="ps2", bufs=4, space="PSUM") as ps2, \
                 tc.tile_pool(name="ps2y", bufs=2, space="PSUM") as ps2y, \
                 tc.tile_pool(name="ps2s", bufs=2, space="PSUM") as ps2s:

                for u in range(B):
                    for qc in range(QC):
                        qs = slice(qc * 512, (qc + 1) * 512)
                        py = ps2y.tile([128, 512], F32, tag="py")
                        psm = ps2s.tile([128, 512], F32, tag="psm")
                        ses = {}
                        for kt in range(KT + 2):
                            if kt < KT:
                                psc = ps2.tile([128, 512], F32, tag="psc")
                                se = p2e.tile([128, 512], BF16, tag="exp")
                                ses[kt] = se
                                ks = slice(kt * 128, (kt + 1) * 128)
                                ks64 = slice(kt * 128, (kt + 1) * 128)
                                nc.tensor.matmul(psc[:], krot[u][:, ks], qrot[u][:, qs], start=True, stop=False)
                                nc.tensor.matmul(psc[:], tfk[u][:, ks64], tfq[u][:, qs], start=False, stop=True)
                                nc.scalar.activation(se[:], psc[:], AF.Exp)
                            if kt >= 2:
                                ktp = kt - 2
                                se = ses.pop(ktp)
                                nc.tensor.matmul(psm[:], tones[:], se[:], start=(ktp == 0), stop=(ktp == KT - 1))
                                nc.tensor.matmul(py[:], vnat[u][:, ktp * 128:(ktp + 1) * 128], se[:],
                                                 start=(ktp == 0), stop=(ktp == KT - 1))
                        rbs = p2r.tile([128, 512], F32, tag="rbs")
                        nc.vector.reciprocal(rbs[:], psm[:])
                        ynrm = p2r.tile([128, 512], BF16, tag="ynrm")
                        nc.vector.tensor_mul(ynrm[:], py[:], rbs[:])
                        nc.sync.dma_start(out=a2a_in[u][2 * qc], in_=ynrm[:, 0:OWN])
                        nc.sync.dma_start(out=a2a_in[u][2 * qc + 1], in_=ynrm[:, OWN:512])
                    nc.gpsimd.collective_compute(
                        "AllToAll", mybir.AluOpType.bypass,
                        ins=[a2a_in[u].opt()], outs=[a2a_out[u].opt()],
                        replica_groups=[list(range(N_CORES))],
                    )

            # ---------------- Phase 3: output projection per A2A round
            with tc.tile_pool(name="ph3", bufs=1) as p3, \
                 tc.tile_pool(name="ph3s", bufs=4) as p3s, \
                 tc.tile_pool(name="ps3", bufs=3, space="PSUM") as ps3:

                for r in range(B):
                    ya = []
                    for dt in range(N_CORES):
                        t = p3.tile([128, OWN], BF16, tag=f"ya{r}_{dt}")
                        nc.sync.dma_start(out=t[:], in_=a2a_out[r][dt])
                        ya.append(t)
                    for tl in range(2):
                        tt = r * 2 + tl
                        for nch in range(2):
                            po = ps3.tile([128, 512], F32, tag="po")
                            for dt in range(CT):
                                nc.tensor.matmul(po[:], ya[dt][:, tl * 128:(tl + 1) * 128],
                                                 two[:, dt * D + nch * 512: dt * D + (nch + 1) * 512],
                                                 start=(dt == 0), stop=(dt == CT - 1))
                            so = p3s.tile([128, 512], F32, tag="so")
                            nc.vector.scalar_tensor_tensor(
                                out=so[:], in0=po[:], scalar=tmask[:, tt:tt + 1],
                                in1=tin1[:, tt * D + nch * 512: tt * D + (nch + 1) * 512],
                                op0=mybir.AluOpType.mult, op1=mybir.AluOpType.add)
                            nc.sync.dma_start(out=out[tt * 128:(tt + 1) * 128, nch * 512:(nch + 1) * 512], in_=so[:])

    _split_multi_waits(nc)
    return nc


def _split_multi_waits(nc):
    """This walrus build encodes at most one sync-wait per instruction; hoist
    extras onto preceding NoOps.  For the kernel-tail drain (many DMA-queue
    waits, followed by an all-engine barrier) spread the NoOps across all
    engines so the waits poll in parallel; elsewhere keep them on the same
    engine to preserve ordering semantics."""
    engs = [mybir.EngineType.SP, mybir.EngineType.Activation, mybir.EngineType.DVE,
            mybir.EngineType.PE, mybir.EngineType.Pool]
    for f in nc.m.functions:
        for bb in f.blocks:
            new_insts = []
            for inst in bb.instructions:
                si = inst.sync_info
                if si is not None and si.on_wait and len(si.on_wait) > 1:
                    waits = list(si.on_wait)
                    distribute = (type(inst).__name__ == "InstDrain"
                                  and len(waits) > 3)
                    for j, w in enumerate(waits[:-1]):
                        eng = engs[j % len(engs)] if distribute else inst.engine
                        new_insts.append(mybir.InstNoOp(
                            name=f"{inst.name}_wsplit{j}", ins=[], outs=[],
                            engine=eng,
                            sync_info=mybir.SyncInfo(on_wait=[w], on_update=[])))
                    si.on_wait = [waits[-1]]
                new_insts.append(inst)
            bb.instructions = new_insts


def _prep_inputs(x, coords, update_mask, Wqkv, bqkv, Wo, bo, W_rope, W_fb,
                 beta_cos, beta_sin):
    """Per-core input maps (host-side layout + tiny trig tables)."""
    f32 = np.float32
    bf16 = ml_dtypes.bfloat16
    x = np.asarray(x, f32)
    coords = np.asarray(coords, f32)
    update_mask = np.asarray(update_mask)
    Wqkv = np.asarray(Wqkv, f32)
    bqkv = np.asarray(bqkv, f32)
    Wo = np.ascontiguousarray(np.asarray(Wo, f32))
    bo = np.asarray(bo, f32)
    W_rope = np.asarray(W_rope, f32)
    W_fb = np.asarray(W_fb, f32)
    beta_cos = np.asarray(beta_cos, f32)
    beta_sin = np.asarray(beta_sin, f32)

    xf = x.reshape(BT, D)
    xT = np.ascontiguousarray(xf.T.astype(bf16))

    # split-half channel order: evens then odds
    perm = np.concatenate([np.arange(0, HD, 2), np.arange(1, HD, 2)])
    inv_scale = f32(1.0 / math.sqrt(HD))

    # rope tables per batch: theta[m, t]; A=[cos;cos], B=[-sin;sin]
    At = np.empty((B, 128, T), bf16)
    Bt = np.empty((B, 128, T), bf16)
    fkT = np.empty((B, 64, T), bf16)
    fqT = np.empty((B, 64, T), bf16)
    for b in range(B):
        c1 = coords[b, :, 0].astype(np.float64)
        th = (W_rope[:, 0:1].astype(np.float64) * c1[None, :])
        cth = np.cos(th).astype(f32)
        sth = np.sin(th).astype(f32)
        At[b] = np.concatenate([cth, cth], axis=0).astype(bf16)
        Bt[b] = np.concatenate([-sth, sth], axis=0).astype(bf16)
        S = (W_fb[:, 0:1].astype(np.float64) * c1[None, :])
        cS = np.cos(S).astype(f32)
        sS = np.sin(S).astype(f32)
        fkT[b] = np.concatenate([cS, sS], axis=0).astype(bf16)
        fqT[b] = np.concatenate([cS * beta_cos[:, None] + sS * beta_sin[:, None],
                                 sS * beta_cos[:, None] - cS * beta_sin[:, None]],
                                axis=0).astype(bf16)

    pswp = np.zeros((128, 128), bf16)
    for i in range(128):
        pswp[(i + 64) % 128, i] = 1.0   # lhsT: (P^T x)[i] = x[(i+64)%128]

    ident_np = np.eye(128, dtype=bf16)
    onesq = np.ones((128, 128), bf16)

    # wo packed: block dt holds Wo rows [dt*128:(dt+1)*128, :]
    wo_pack = np.empty((128, CT * D), bf16)
    for dt in range(CT):
        wo_pack[:, dt * D:(dt + 1) * D] = Wo[dt * 128:(dt + 1) * 128, :].astype(bf16)

    mask_f = update_mask.reshape(BT).astype(f32)

    in_maps = []
    for c in range(N_CORES):
        h = c
        wq_h = (Wqkv[:, h * HD:(h + 1) * HD][:, perm] * inv_scale)
        wk_h = Wqkv[:, D + h * HD:D + (h + 1) * HD][:, perm]
        wv_h = Wqkv[:, 2 * D + h * HD:2 * D + (h + 1) * HD]
        # pack [128, CT*HD]: block ct = w[ct*128:(ct+1)*128, :]
        def pack(w):
            p = np.empty((128, CT * HD), bf16)
            for ct in range(CT):
                p[:, ct * HD:(ct + 1) * HD] = w[ct * 128:(ct + 1) * 128, :].astype(bf16)
            return p
        bq_h = (bqkv[h * HD:(h + 1) * HD][perm] * inv_scale).reshape(HD, 1)
        bk_h = bqkv[D + h * HD:D + (h + 1) * HD][perm].reshape(HD, 1)
        bv_h = bqkv[2 * D + h * HD:2 * D + (h + 1) * HD].reshape(HD, 1)
        # owned rows: batch-0 tokens [c*OWN:(c+1)*OWN] then batch-1 same slice
        rows = np.r_[np.arange(c * OWN, (c + 1) * OWN),
                     np.arange(T + c * OWN, T + (c + 1) * OWN)]
        mrows = mask_f[rows].reshape(ROWS, 1)
        in1 = mrows * bo[None, :] + (1.0 - mrows) * xf[rows]
        in_maps.append(dict(
            xT=xT, wq=pack(wq_h), wk=pack(wk_h), wv=pack(wv_h),
            bq=np.ascontiguousarray(bq_h, f32), bk=np.ascontiguousarray(bk_h, f32),
            bv=np.ascontiguousarray(bv_h, f32),
            At=At, Bt=Bt, fk=fkT, fq=fqT, pswp=pswp,
            identity=ident_np, onesq=onesq, wo=wo_pack,
            maskc=np.ascontiguousarray(mrows, f32),
            in1m=np.ascontiguousarray(in1, f32),
        ))
    return in_maps


_NC_CACHE = None


def _get_nc():
    global _NC_CACHE
    if _NC_CACHE is None:
        _NC_CACHE = _build_nc()
    return _NC_CACHE


def run(trace=False, **inputs):
    nc = _get_nc()
    in_maps = _prep_inputs(**inputs)
    res = run_bass_kernel_spmd(nc, in_maps, core_ids=list(range(N_CORES)),
                               trace=trace)
    full = np.empty((B, T, D), np.float32)
    for c in range(N_CORES):
        o = res.results[c]["out"]
        full[0, c * OWN:(c + 1) * OWN] = o[0:OWN]
        full[1, c * OWN:(c + 1) * OWN] = o[OWN:ROWS]
    return full, res


def kernel(**inputs) -> np.ndarray:
    full, _ = run(trace=False, **inputs)
    return full
